# revision 25
# baseline (speedup 1.0000x reference)
"""Trainium2 Bass kernel for the VAE-style loss function.

Computes, from full inputs
    x, x_out: [256, 3, 128, 128] f32
    y:        [256, 7]  f32 (integer labels 0..9 with NaN = unlabeled)
    mu:       [256, 32] f32
    disc_pos: [10]      f32
the three scalars (recon, kld, recon + kld) exactly as the reference:
    recon   = |x - x_out|.sum(axis=(1,2,3)).mean()
    kld_d   = where(isnan(y_d), min_p (mu_d - pos_p)^2, (mu_d - pos[y_d])^2).mean(0).sum()
    kld_l   = where(isnan(y_l), relu(|mu_l| - 10)^2, (mu_l - y_l)^2).sum(1).mean()
    kld     = kld_d + kld_l

Strategy: pure data parallel over the batch dim across 8 NeuronCores.
Each core reduces its 32-sample slice to three partial sums (one SPMD
program, per-core input slices), and the host sums the 8 x 3 partials
and divides by 256.

Schedule notes (from trace analysis of the first version):
  - the 6 big-chunk DMAs are issued back-to-back on the sync HWDGE ring
    so they drain continuously;
  - the smalls DMA goes on the scalar HWDGE ring (a [32,236B] DMA took
    ~7us of descriptor generation and was blocking the big-chunk issues
    when it sat on the sync ring); rows padded to 1 KiB;
  - the kld math is vectorized via host-side replication packing: the
    per-dim codebook loops become ~19 wide DVE ops instead of ~90 tiny
    ones (which previously added an ~8us serial tail).
"""

import numpy as np

import concourse.bass as bass
import concourse.mybir as mybir
import concourse.bacc as bacc
import concourse.tile as tile


F32 = mybir.dt.float32
ALU = mybir.AluOpType
AXIS = mybir.AxisListType

N_CORES = 8
B = 256
BL = B // N_CORES          # 32 samples per core
P = 128                    # SBUF partitions
TOT = BL * 3 * 128 * 128   # 1572864 elements per big tensor per core
FREE = TOT // P            # 12288 elements per partition
NCHUNK = 6
CH = FREE // NCHUNK        # 2048
ND = 3                     # discrete dims
NL = 4                     # linear dims
NPOS = 10                  # codebook positions


# smalls packing: one [BL, 256] f32 tensor (1 KiB rows), column map:
#   [0:30)    mu_rep30   mu[:, d] replicated x10 (d-major)
#   [30:60)   posb30     disc_pos tiled x3 (becomes dist scratch)
#   [60:100)  iota40     arange(10) tiled x4 (for the fused sel mul)
#   [100:130) iota30     arange(10) tiled x3
#   [130:170) iota40b    arange(10) tiled x4 (adjacent to iota30 -> iota70)
#   [170:240) y_rep70    y[:, d] replicated x10, disc then linear
#   [240:244) mu_l
#   [244:248) y_l
#   [248:255) y7         y[:, 0:7]
#   [255:256) pad
SM_W = 256


def build_module():
    nc = bacc.Bacc(
        "TRN2",
        target_bir_lowering=False,
        debug=False,
        num_devices=N_CORES,
        enable_partition_id=False,
    )
    # x and x_out stacked host-side so each chunk is a single DMA.
    xc = nc.dram_tensor("xc", [2, TOT], F32, kind="ExternalInput")
    sm = nc.dram_tensor("smalls", [BL, SM_W], F32, kind="ExternalInput")
    out = nc.dram_tensor("out", [1, 3], F32, kind="ExternalOutput")

    # [2, TOT] -> [p, 2, n]: partition-major within each half
    xcf = xc.ap().rearrange("h (p n) -> p h n", p=P)

    with tile.TileContext(nc) as tc:
        with (
            tc.tile_pool(name="big", bufs=NCHUNK) as bp,
            tc.tile_pool(name="acc", bufs=1) as cp,
            tc.tile_pool(name="small", bufs=1) as sp,
            tc.tile_pool(name="work", bufs=1) as wp,
            tc.tile_pool(name="psum", bufs=1, space="PSUM") as pp,
        ):
            # ---------------- all DMAs issued first ----------------
            acc = cp.tile([P, NCHUNK], F32)
            xts = []
            for i in range(NCHUNK):
                xt = bp.tile([P, 2, CH], F32, tag="xt")
                nc.sync.dma_start(out=xt[:], in_=xcf[:, :, i * CH : (i + 1) * CH])
                xts.append(xt)

            sm_t = sp.tile([BL, SM_W], F32)
            nc.scalar.dma_start(out=sm_t[:], in_=sm.ap())

            # stk collects per-partition partials: col0 recon (128 rows),
            # col1 disc, col2 lin (32 rows each, rest zero)
            stk = cp.tile([P, 3], F32)
            nc.vector.memset(stk[:], 0.0)
            ones_t = sp.tile([P, 1], F32)
            nc.vector.memset(ones_t[:], 1.0)

            # ---------------- vectorized kld (smalls) ----------------
            MU30 = sm_t[:, 0:30]
            DI30 = sm_t[:, 30:60]      # posb30 -> dist scratch (in place)
            SEL70 = sm_t[:, 30:100]    # [dist | iota40]
            IO70 = sm_t[:, 100:170]    # [iota30 | iota40b]
            YR70 = sm_t[:, 170:240]
            MUL4 = sm_t[:, 240:244]
            YL4 = sm_t[:, 244:248]
            Y7 = sm_t[:, 248:255]

            oh70 = wp.tile([BL, 70], F32)
            nc.vector.tensor_tensor(oh70[:], IO70, YR70, ALU.is_equal)
            # dist = (pos - mu)^2, in place over posb30
            nc.vector.tensor_sub(DI30, DI30, MU30)
            nc.vector.tensor_mul(DI30, DI30, DI30)
            # sel70 = oh * [dist | iota40]
            sel = wp.tile([BL, 70], F32)
            nc.vector.tensor_mul(sel[:], oh70[:], SEL70)
            # per-dim sums of sel: [lab_d (3) | ysafe_l (4)]
            R7 = wp.tile([BL, 7], F32)
            nc.vector.tensor_reduce(
                R7[:], sel[:].rearrange("p (a b) -> p a b", b=NPOS), AXIS.X, ALU.add
            )
            # unlabeled disc: min over codebook positions
            U3 = wp.tile([BL, ND], F32)
            nc.vector.tensor_reduce(
                U3[:], DI30.rearrange("p (a b) -> p a b", b=NPOS), AXIS.X, ALU.min
            )
            # labeled mask for all 7 dims (NaN != NaN)
            EQ7 = wp.tile([BL, ND + NL], F32)
            nc.vector.tensor_tensor(EQ7[:], Y7, Y7, ALU.is_equal)

            # disc: sel = unl + (lab - unl) * eq, summed over d
            T3 = wp.tile([BL, ND], F32)
            nc.vector.tensor_sub(T3[:], R7[:, 0:ND], U3[:])
            nc.vector.tensor_mul(T3[:], T3[:], EQ7[:, 0:ND])
            nc.vector.tensor_add(T3[:], T3[:], U3[:])
            nc.vector.tensor_reduce(stk[0:BL, 1:2], T3[:], AXIS.X, ALU.add)

            # linear: lab = (mu - ysafe)^2 ; n = relu(|mu| - 10)^2
            D4 = wp.tile([BL, NL], F32)
            nc.vector.tensor_sub(D4[:], MUL4, R7[:, ND : ND + NL])
            L4 = wp.tile([BL, NL], F32)
            nc.vector.tensor_mul(L4[:], D4[:], D4[:])
            N4 = wp.tile([BL, NL], F32)
            nc.vector.tensor_scalar(N4[:], MUL4, -1.0, None, ALU.mult)
            A4 = wp.tile([BL, NL], F32)
            nc.vector.tensor_max(A4[:], MUL4, N4[:])
            nc.vector.tensor_scalar(A4[:], A4[:], -10.0, 0.0, ALU.add, ALU.max)
            nc.vector.tensor_mul(A4[:], A4[:], A4[:])
            nc.vector.tensor_sub(L4[:], L4[:], A4[:])
            nc.vector.tensor_mul(L4[:], L4[:], EQ7[:, ND : ND + NL])
            nc.vector.tensor_add(L4[:], L4[:], A4[:])
            nc.vector.tensor_reduce(stk[0:BL, 2:3], L4[:], AXIS.X, ALU.add)

            # ---------------- recon: sum |x - x_out| ----------------
            for i, xt in enumerate(xts):
                nc.vector.tensor_sub(xt[:, 0, :], xt[:, 0, :], xt[:, 1, :])
                nc.vector.tensor_reduce(
                    acc[:, i : i + 1],
                    xt[:, 0, :],
                    AXIS.X,
                    ALU.add,
                    apply_absolute_value=True,
                )
            nc.vector.tensor_reduce(stk[:, 0:1], acc[:], AXIS.X, ALU.add)

            # ---------------- partial-sum outputs ----------------
            # partition-reduce all three columns at once: ones.T @ stk -> [1,3]
            ps = pp.tile([1, 3], F32)
            nc.tensor.matmul(ps[:], ones_t[:], stk[:], start=True, stop=True)
            res = sp.tile([1, 3], F32)
            nc.vector.tensor_copy(res[:], ps[:])
            nc.sync.dma_start(out=out.ap(), in_=res[:])

    nc.compile()
    return nc


_NC_CACHE = None


def _get_module():
    global _NC_CACHE
    if _NC_CACHE is None:
        _NC_CACHE = build_module()
    return _NC_CACHE


def make_in_maps(x, x_out, y, mu, disc_pos):
    x = np.ascontiguousarray(x, dtype=np.float32)
    x_out = np.ascontiguousarray(x_out, dtype=np.float32)
    y = np.ascontiguousarray(y, dtype=np.float32)
    mu = np.ascontiguousarray(mu, dtype=np.float32)
    disc_pos = np.ascontiguousarray(disc_pos, dtype=np.float32)

    iota10 = np.arange(NPOS, dtype=np.float32)
    posb30 = np.tile(disc_pos, (BL, ND))
    iota30 = np.tile(iota10, (BL, ND))
    iota40 = np.tile(iota10, (BL, NL))

    in_maps = []
    for i in range(N_CORES):
        s = slice(i * BL, (i + 1) * BL)
        xcore = np.empty((2, TOT), dtype=np.float32)
        xcore[0] = x[s].reshape(-1)
        xcore[1] = x_out[s].reshape(-1)

        mu_s, y_s = mu[s], y[s]
        mu_d, mu_l = mu_s[:, :ND], mu_s[:, ND : ND + NL]
        y_d, y_l = y_s[:, :ND], y_s[:, ND : ND + NL]
        mu_rep30 = np.repeat(mu_d, NPOS, axis=1)
        y_rep70 = np.concatenate(
            [np.repeat(y_d, NPOS, axis=1), np.repeat(y_l, NPOS, axis=1)], axis=1
        )
        smalls = np.concatenate(
            [
                mu_rep30,           # [0:30)
                posb30,             # [30:60)
                iota40,             # [60:100)
                iota30,             # [100:130)
                iota40,             # [130:170)
                y_rep70,            # [170:240)
                mu_l,               # [240:244)
                y_l,                # [244:248)
                y_s[:, : ND + NL],  # [248:255)
                np.zeros((BL, 1), dtype=np.float32),  # pad to 256
            ],
            axis=1,
        ).astype(np.float32)
        assert smalls.shape == (BL, SM_W)
        in_maps.append({"xc": xcore, "smalls": smalls})
    return in_maps


def combine_partials(partials):
    """partials: [8, 1, 3] (or [8, 3]) per-core sums -> full (3,) output."""
    p = np.asarray(partials, dtype=np.float64).reshape(N_CORES, 3)
    s = p.sum(axis=0) / B
    recon = s[0]
    kld = s[1] + s[2]
    return np.array([recon, kld, recon + kld], dtype=np.float32)


def run_spmd(x, x_out, y, mu, disc_pos, trace=False, **kw):
    from concourse.bass_utils import run_bass_kernel_spmd

    nc = _get_module()
    in_maps = make_in_maps(x, x_out, y, mu, disc_pos)
    r = run_bass_kernel_spmd(nc, in_maps, list(range(N_CORES)), trace=trace, **kw)
    partials = [r.results[i]["out"] for i in range(N_CORES)]
    return combine_partials(partials), r


def kernel(x, x_out, y, mu, disc_pos):
    out, _ = run_spmd(x, x_out, y, mu, disc_pos)
    return out


if __name__ == "__main__":
    nc = build_module()
    print("module built ok")


# revision 28
# speedup vs baseline: 1.1048x; 1.1048x over previous
"""Trainium2 Bass kernel for the VAE-style loss function.

Computes, from full inputs
    x, x_out: [256, 3, 128, 128] f32
    y:        [256, 7]  f32 (integer labels 0..9 with NaN = unlabeled)
    mu:       [256, 32] f32
    disc_pos: [10]      f32
the three scalars (recon, kld, recon + kld) exactly as the reference:
    recon   = |x - x_out|.sum(axis=(1,2,3)).mean()
    kld_d   = where(isnan(y_d), min_p (mu_d - pos_p)^2, (mu_d - pos[y_d])^2).mean(0).sum()
    kld_l   = where(isnan(y_l), relu(|mu_l| - 10)^2, (mu_l - y_l)^2).sum(1).mean()
    kld     = kld_d + kld_l

Strategy: pure data parallel over the batch dim across 8 NeuronCores.
Each core reduces its 32-sample slice to three partial sums (one SPMD
program, per-core input slices), and the host sums the 8 x 3 partials
and divides by 256.

Schedule notes (from trace analysis of the first version):
  - the 6 big-chunk DMAs are issued back-to-back on the sync HWDGE ring
    so they drain continuously;
  - the smalls DMA goes on the scalar HWDGE ring (a [32,236B] DMA took
    ~7us of descriptor generation and was blocking the big-chunk issues
    when it sat on the sync ring); rows padded to 1 KiB;
  - the kld math is vectorized via host-side replication packing: the
    per-dim codebook loops become ~19 wide DVE ops instead of ~90 tiny
    ones (which previously added an ~8us serial tail).
"""

import numpy as np

import concourse.bass as bass
import concourse.mybir as mybir
import concourse.bacc as bacc
import concourse.tile as tile


F32 = mybir.dt.float32
ALU = mybir.AluOpType
AXIS = mybir.AxisListType

N_CORES = 8
B = 256
BL = B // N_CORES          # 32 samples per core
P = 128                    # SBUF partitions
TOT = BL * 3 * 128 * 128   # 1572864 elements per big tensor per core
FREE = TOT // P            # 12288 elements per partition
NCHUNK = 6
CH = FREE // NCHUNK        # 2048
ND = 3                     # discrete dims
NL = 4                     # linear dims
NPOS = 10                  # codebook positions


# smalls packing: one [BL, 256] f32 tensor (1 KiB rows), column map:
#   [0:30)    mu_rep30   mu[:, d] replicated x10 (d-major)
#   [30:60)   posb30     disc_pos tiled x3 (becomes dist scratch)
#   [60:100)  iota40     arange(10) tiled x4 (for the fused sel mul)
#   [100:130) iota30     arange(10) tiled x3
#   [130:170) iota40b    arange(10) tiled x4 (adjacent to iota30 -> iota70)
#   [170:240) y_rep70    y[:, d] replicated x10, disc then linear
#   [240:244) mu_l
#   [244:248) y_l
#   [248:255) y7         y[:, 0:7]
#   [255:256) pad
SM_W = 256


def build_module():
    nc = bacc.Bacc(
        "TRN2", target_bir_lowering=False, debug=False, num_devices=N_CORES
    )
    # x and x_out stacked host-side so each chunk is a single DMA.
    xc = nc.dram_tensor("xc", [2, TOT], F32, kind="ExternalInput")
    sm = nc.dram_tensor("smalls", [BL, SM_W], F32, kind="ExternalInput")
    out = nc.dram_tensor("out", [1, 3], F32, kind="ExternalOutput")
    # Unused: perturbs the allocation map to test placement sensitivity.
    nc.dram_tensor("pad0", [1, 1], F32, kind="ExternalInput")

    # [2, TOT] -> [p, 2, n]: partition-major within each half
    xcf = xc.ap().rearrange("h (p n) -> p h n", p=P)

    with tile.TileContext(nc) as tc:
        with (
            tc.tile_pool(name="big", bufs=NCHUNK) as bp,
            tc.tile_pool(name="acc", bufs=1) as cp,
            tc.tile_pool(name="small", bufs=1) as sp,
            tc.tile_pool(name="work", bufs=1) as wp,
            tc.tile_pool(name="psum", bufs=1, space="PSUM") as pp,
        ):
            # ---------------- all DMAs issued first ----------------
            acc = cp.tile([P, NCHUNK], F32)
            xts = []
            for i in range(NCHUNK):
                xt = bp.tile([P, 2, CH], F32, tag="xt")
                nc.sync.dma_start(out=xt[:], in_=xcf[:, :, i * CH : (i + 1) * CH])
                xts.append(xt)

            sm_t = sp.tile([BL, SM_W], F32)
            nc.scalar.dma_start(out=sm_t[:], in_=sm.ap())

            # stk collects per-partition partials: col0 recon (128 rows),
            # col1 disc, col2 lin (32 rows each, rest zero)
            stk = cp.tile([P, 3], F32)
            nc.vector.memset(stk[:], 0.0)
            ones_t = sp.tile([P, 1], F32)
            nc.vector.memset(ones_t[:], 1.0)

            # ---------------- vectorized kld (smalls) ----------------
            MU30 = sm_t[:, 0:30]
            DI30 = sm_t[:, 30:60]      # posb30 -> dist scratch (in place)
            SEL70 = sm_t[:, 30:100]    # [dist | iota40]
            IO70 = sm_t[:, 100:170]    # [iota30 | iota40b]
            YR70 = sm_t[:, 170:240]
            MUL4 = sm_t[:, 240:244]
            YL4 = sm_t[:, 244:248]
            Y7 = sm_t[:, 248:255]

            oh70 = wp.tile([BL, 70], F32)
            nc.vector.tensor_tensor(oh70[:], IO70, YR70, ALU.is_equal)
            # dist = (pos - mu)^2, in place over posb30
            nc.vector.tensor_sub(DI30, DI30, MU30)
            nc.vector.tensor_mul(DI30, DI30, DI30)
            # sel70 = oh * [dist | iota40]
            sel = wp.tile([BL, 70], F32)
            nc.vector.tensor_mul(sel[:], oh70[:], SEL70)
            # per-dim sums of sel: [lab_d (3) | ysafe_l (4)]
            R7 = wp.tile([BL, 7], F32)
            nc.vector.tensor_reduce(
                R7[:], sel[:].rearrange("p (a b) -> p a b", b=NPOS), AXIS.X, ALU.add
            )
            # unlabeled disc: min over codebook positions
            U3 = wp.tile([BL, ND], F32)
            nc.vector.tensor_reduce(
                U3[:], DI30.rearrange("p (a b) -> p a b", b=NPOS), AXIS.X, ALU.min
            )
            # labeled mask for all 7 dims (NaN != NaN)
            EQ7 = wp.tile([BL, ND + NL], F32)
            nc.vector.tensor_tensor(EQ7[:], Y7, Y7, ALU.is_equal)

            # disc: sel = unl + (lab - unl) * eq, summed over d
            T3 = wp.tile([BL, ND], F32)
            nc.vector.tensor_sub(T3[:], R7[:, 0:ND], U3[:])
            nc.vector.tensor_mul(T3[:], T3[:], EQ7[:, 0:ND])
            nc.vector.tensor_add(T3[:], T3[:], U3[:])
            nc.vector.tensor_reduce(stk[0:BL, 1:2], T3[:], AXIS.X, ALU.add)

            # linear: lab = (mu - ysafe)^2 ; n = relu(|mu| - 10)^2
            D4 = wp.tile([BL, NL], F32)
            nc.vector.tensor_sub(D4[:], MUL4, R7[:, ND : ND + NL])
            L4 = wp.tile([BL, NL], F32)
            nc.vector.tensor_mul(L4[:], D4[:], D4[:])
            N4 = wp.tile([BL, NL], F32)
            nc.vector.tensor_scalar(N4[:], MUL4, -1.0, None, ALU.mult)
            A4 = wp.tile([BL, NL], F32)
            nc.vector.tensor_max(A4[:], MUL4, N4[:])
            nc.vector.tensor_scalar(A4[:], A4[:], -10.0, 0.0, ALU.add, ALU.max)
            nc.vector.tensor_mul(A4[:], A4[:], A4[:])
            nc.vector.tensor_sub(L4[:], L4[:], A4[:])
            nc.vector.tensor_mul(L4[:], L4[:], EQ7[:, ND : ND + NL])
            nc.vector.tensor_add(L4[:], L4[:], A4[:])
            nc.vector.tensor_reduce(stk[0:BL, 2:3], L4[:], AXIS.X, ALU.add)

            # ---------------- recon: sum |x - x_out| ----------------
            for i, xt in enumerate(xts):
                nc.vector.tensor_sub(xt[:, 0, :], xt[:, 0, :], xt[:, 1, :])
                nc.vector.tensor_reduce(
                    acc[:, i : i + 1],
                    xt[:, 0, :],
                    AXIS.X,
                    ALU.add,
                    apply_absolute_value=True,
                )
            nc.vector.tensor_reduce(stk[:, 0:1], acc[:], AXIS.X, ALU.add)

            # ---------------- partial-sum outputs ----------------
            # partition-reduce all three columns at once: ones.T @ stk -> [1,3]
            ps = pp.tile([1, 3], F32)
            nc.tensor.matmul(ps[:], ones_t[:], stk[:], start=True, stop=True)
            res = sp.tile([1, 3], F32)
            nc.vector.tensor_copy(res[:], ps[:])
            nc.sync.dma_start(out=out.ap(), in_=res[:])

    nc.compile()
    return nc


_NC_CACHE = None


def _get_module():
    global _NC_CACHE
    if _NC_CACHE is None:
        _NC_CACHE = build_module()
    return _NC_CACHE


def make_in_maps(x, x_out, y, mu, disc_pos):
    x = np.ascontiguousarray(x, dtype=np.float32)
    x_out = np.ascontiguousarray(x_out, dtype=np.float32)
    y = np.ascontiguousarray(y, dtype=np.float32)
    mu = np.ascontiguousarray(mu, dtype=np.float32)
    disc_pos = np.ascontiguousarray(disc_pos, dtype=np.float32)

    iota10 = np.arange(NPOS, dtype=np.float32)
    posb30 = np.tile(disc_pos, (BL, ND))
    iota30 = np.tile(iota10, (BL, ND))
    iota40 = np.tile(iota10, (BL, NL))

    in_maps = []
    for i in range(N_CORES):
        s = slice(i * BL, (i + 1) * BL)
        xcore = np.empty((2, TOT), dtype=np.float32)
        xcore[0] = x[s].reshape(-1)
        xcore[1] = x_out[s].reshape(-1)

        mu_s, y_s = mu[s], y[s]
        mu_d, mu_l = mu_s[:, :ND], mu_s[:, ND : ND + NL]
        y_d, y_l = y_s[:, :ND], y_s[:, ND : ND + NL]
        mu_rep30 = np.repeat(mu_d, NPOS, axis=1)
        y_rep70 = np.concatenate(
            [np.repeat(y_d, NPOS, axis=1), np.repeat(y_l, NPOS, axis=1)], axis=1
        )
        smalls = np.concatenate(
            [
                mu_rep30,           # [0:30)
                posb30,             # [30:60)
                iota40,             # [60:100)
                iota30,             # [100:130)
                iota40,             # [130:170)
                y_rep70,            # [170:240)
                mu_l,               # [240:244)
                y_l,                # [244:248)
                y_s[:, : ND + NL],  # [248:255)
                np.zeros((BL, 1), dtype=np.float32),  # pad to 256
            ],
            axis=1,
        ).astype(np.float32)
        assert smalls.shape == (BL, SM_W)
        in_maps.append(
            {
                "xc": xcore,
                "smalls": smalls,
                "pad0": np.zeros((1, 1), dtype=np.float32),
            }
        )
    return in_maps


def combine_partials(partials):
    """partials: [8, 1, 3] (or [8, 3]) per-core sums -> full (3,) output."""
    p = np.asarray(partials, dtype=np.float64).reshape(N_CORES, 3)
    s = p.sum(axis=0) / B
    recon = s[0]
    kld = s[1] + s[2]
    return np.array([recon, kld, recon + kld], dtype=np.float32)


def run_spmd(x, x_out, y, mu, disc_pos, trace=False, **kw):
    from concourse.bass_utils import run_bass_kernel_spmd

    nc = _get_module()
    in_maps = make_in_maps(x, x_out, y, mu, disc_pos)
    r = run_bass_kernel_spmd(nc, in_maps, list(range(N_CORES)), trace=trace, **kw)
    partials = [r.results[i]["out"] for i in range(N_CORES)]
    return combine_partials(partials), r


def kernel(x, x_out, y, mu, disc_pos):
    out, _ = run_spmd(x, x_out, y, mu, disc_pos)
    return out


if __name__ == "__main__":
    nc = build_module()
    print("module built ok")


# revision 32
# speedup vs baseline: 1.1279x; 1.0209x over previous
"""Trainium2 Bass kernel for the VAE-style loss function.

Computes, from full inputs
    x, x_out: [256, 3, 128, 128] f32
    y:        [256, 7]  f32 (integer labels 0..9 with NaN = unlabeled)
    mu:       [256, 32] f32
    disc_pos: [10]      f32
the three scalars (recon, kld, recon + kld) exactly as the reference:
    recon   = |x - x_out|.sum(axis=(1,2,3)).mean()
    kld_d   = where(isnan(y_d), min_p (mu_d - pos_p)^2, (mu_d - pos[y_d])^2).mean(0).sum()
    kld_l   = where(isnan(y_l), relu(|mu_l| - 10)^2, (mu_l - y_l)^2).sum(1).mean()
    kld     = kld_d + kld_l

Strategy: pure data parallel over the batch dim across 8 NeuronCores.
Each core reduces its 32-sample slice to three partial sums (one SPMD
program, per-core input slices), and the host sums the 8 x 3 partials
and divides by 256.

Schedule notes (from trace analysis of the first version):
  - the 6 big-chunk DMAs are issued back-to-back on the sync HWDGE ring
    so they drain continuously;
  - the smalls DMA goes on the scalar HWDGE ring (a [32,236B] DMA took
    ~7us of descriptor generation and was blocking the big-chunk issues
    when it sat on the sync ring); rows padded to 1 KiB;
  - the kld math is vectorized via host-side replication packing: the
    per-dim codebook loops become ~19 wide DVE ops instead of ~90 tiny
    ones (which previously added an ~8us serial tail).
"""

import numpy as np

import concourse.bass as bass
import concourse.mybir as mybir
import concourse.bacc as bacc
import concourse.tile as tile


F32 = mybir.dt.float32
ALU = mybir.AluOpType
AXIS = mybir.AxisListType

N_CORES = 8
B = 256
BL = B // N_CORES          # 32 samples per core
P = 128                    # SBUF partitions
TOT = BL * 3 * 128 * 128   # 1572864 elements per big tensor per core
FREE = TOT // P            # 12288 elements per partition
NCHUNK = 6
CH = FREE // NCHUNK        # 2048
ND = 3                     # discrete dims
NL = 4                     # linear dims
NPOS = 10                  # codebook positions


# smalls packing: one [BL, 256] f32 tensor (1 KiB rows), column map:
#   [0:30)    mu_rep30   mu[:, d] replicated x10 (d-major)
#   [30:60)   posb30     disc_pos tiled x3 (becomes dist scratch)
#   [60:100)  iota40     arange(10) tiled x4 (for the fused sel mul)
#   [100:130) iota30     arange(10) tiled x3
#   [130:170) iota40b    arange(10) tiled x4 (adjacent to iota30 -> iota70)
#   [170:240) y_rep70    y[:, d] replicated x10, disc then linear
#   [240:244) mu_l
#   [244:248) y_l
#   [248:255) y7         y[:, 0:7]
#   [255:256) pad
SM_W = 256


def build_module():
    nc = bacc.Bacc(
        "TRN2", target_bir_lowering=False, debug=False, num_devices=N_CORES
    )
    # x and x_out stacked host-side so each chunk is a single DMA.
    xc = nc.dram_tensor("xc", [2, TOT], F32, kind="ExternalInput")
    sm = nc.dram_tensor("smalls", [BL, SM_W], F32, kind="ExternalInput")
    out = nc.dram_tensor("out", [1, 3], F32, kind="ExternalOutput")

    # [2, TOT] -> [p, 2, n]: partition-major within each half
    xcf = xc.ap().rearrange("h (p n) -> p h n", p=P)

    with tile.TileContext(nc) as tc:
        with (
            tc.tile_pool(name="big", bufs=NCHUNK) as bp,
            tc.tile_pool(name="acc", bufs=1) as cp,
            tc.tile_pool(name="small", bufs=1) as sp,
            tc.tile_pool(name="work", bufs=1) as wp,
            tc.tile_pool(name="psum", bufs=1, space="PSUM") as pp,
        ):
            # ---------------- all DMAs issued first ----------------
            # acc: recon partials, one col per chunk 0-4 plus two for the
            # halves of the last chunk (ACT-assisted tail below).
            acc = cp.tile([P, NCHUNK + 1], F32)
            xts = []
            for i in range(NCHUNK):
                xt = bp.tile([P, 2, CH], F32, tag="xt")
                nc.sync.dma_start(out=xt[:], in_=xcf[:, :, i * CH : (i + 1) * CH])
                xts.append(xt)

            sm_t = sp.tile([BL, SM_W], F32)
            nc.scalar.dma_start(out=sm_t[:], in_=sm.ap())

            # stk collects per-partition partials: col0 recon (128 rows),
            # col1 disc, col2 lin (32 rows each, rest zero)
            stk = cp.tile([P, 3], F32)
            nc.vector.memset(stk[:], 0.0)
            ones_t = sp.tile([P, 1], F32)
            nc.vector.memset(ones_t[:], 1.0)

            # ---------------- vectorized kld (smalls) ----------------
            MU30 = sm_t[:, 0:30]
            DI30 = sm_t[:, 30:60]      # posb30 -> dist scratch (in place)
            SEL70 = sm_t[:, 30:100]    # [dist | iota40]
            IO70 = sm_t[:, 100:170]    # [iota30 | iota40b]
            YR70 = sm_t[:, 170:240]
            MUL4 = sm_t[:, 240:244]
            YL4 = sm_t[:, 244:248]
            Y7 = sm_t[:, 248:255]

            oh70 = wp.tile([BL, 70], F32)
            nc.vector.tensor_tensor(oh70[:], IO70, YR70, ALU.is_equal)
            # dist = (pos - mu)^2, in place over posb30
            nc.vector.tensor_sub(DI30, DI30, MU30)
            nc.vector.tensor_mul(DI30, DI30, DI30)
            # sel70 = oh * [dist | iota40]
            sel = wp.tile([BL, 70], F32)
            nc.vector.tensor_mul(sel[:], oh70[:], SEL70)
            # per-dim sums of sel: [lab_d (3) | ysafe_l (4)]
            R7 = wp.tile([BL, 7], F32)
            nc.vector.tensor_reduce(
                R7[:], sel[:].rearrange("p (a b) -> p a b", b=NPOS), AXIS.X, ALU.add
            )
            # unlabeled disc: min over codebook positions
            U3 = wp.tile([BL, ND], F32)
            nc.vector.tensor_reduce(
                U3[:], DI30.rearrange("p (a b) -> p a b", b=NPOS), AXIS.X, ALU.min
            )
            # labeled mask for all 7 dims (NaN != NaN)
            EQ7 = wp.tile([BL, ND + NL], F32)
            nc.vector.tensor_tensor(EQ7[:], Y7, Y7, ALU.is_equal)

            # disc: sel = unl + (lab - unl) * eq, summed over d
            T3 = wp.tile([BL, ND], F32)
            nc.vector.tensor_sub(T3[:], R7[:, 0:ND], U3[:])
            nc.vector.tensor_mul(T3[:], T3[:], EQ7[:, 0:ND])
            nc.vector.tensor_add(T3[:], T3[:], U3[:])
            nc.vector.tensor_reduce(stk[0:BL, 1:2], T3[:], AXIS.X, ALU.add)

            # linear: lab = (mu - ysafe)^2 ; n = relu(|mu| - 10)^2
            D4 = wp.tile([BL, NL], F32)
            nc.vector.tensor_sub(D4[:], MUL4, R7[:, ND : ND + NL])
            L4 = wp.tile([BL, NL], F32)
            nc.vector.tensor_mul(L4[:], D4[:], D4[:])
            N4 = wp.tile([BL, NL], F32)
            nc.vector.tensor_scalar(N4[:], MUL4, -1.0, None, ALU.mult)
            A4 = wp.tile([BL, NL], F32)
            nc.vector.tensor_max(A4[:], MUL4, N4[:])
            nc.vector.tensor_scalar(A4[:], A4[:], -10.0, 0.0, ALU.add, ALU.max)
            nc.vector.tensor_mul(A4[:], A4[:], A4[:])
            nc.vector.tensor_sub(L4[:], L4[:], A4[:])
            nc.vector.tensor_mul(L4[:], L4[:], EQ7[:, ND : ND + NL])
            nc.vector.tensor_add(L4[:], L4[:], A4[:])
            nc.vector.tensor_reduce(stk[0:BL, 2:3], L4[:], AXIS.X, ALU.add)

            # ---------------- recon: sum |x - x_out| ----------------
            for i, xt in enumerate(xts[:-1]):
                nc.vector.tensor_sub(xt[:, 0, :], xt[:, 0, :], xt[:, 1, :])
                nc.vector.tensor_reduce(
                    acc[:, i : i + 1],
                    xt[:, 0, :],
                    AXIS.X,
                    ALU.add,
                    apply_absolute_value=True,
                )
            # Last chunk sets the exec tail (its data lands as the DMA
            # stream ends): split it in half, DVE doing the two subs
            # back-to-back while the otherwise-idle ACT engine does the
            # fused abs+accumulate per half.
            xt = xts[-1]
            H = CH // 2
            junk = cp.tile([P, H], F32, tag="junk")
            ABS = mybir.ActivationFunctionType.Abs
            nc.vector.tensor_sub(xt[:, 0, 0:H], xt[:, 0, 0:H], xt[:, 1, 0:H])
            nc.scalar.activation(
                junk[:], xt[:, 0, 0:H], ABS, accum_out=acc[:, NCHUNK - 1 : NCHUNK]
            )
            nc.vector.tensor_sub(xt[:, 0, H:CH], xt[:, 0, H:CH], xt[:, 1, H:CH])
            nc.scalar.activation(
                junk[:], xt[:, 0, H:CH], ABS, accum_out=acc[:, NCHUNK : NCHUNK + 1]
            )
            nc.vector.tensor_reduce(stk[:, 0:1], acc[:], AXIS.X, ALU.add)

            # ---------------- partial-sum outputs ----------------
            # partition-reduce all three columns at once: ones.T @ stk -> [1,3]
            ps = pp.tile([1, 3], F32)
            nc.tensor.matmul(ps[:], ones_t[:], stk[:], start=True, stop=True)
            res = sp.tile([1, 3], F32)
            nc.vector.tensor_copy(res[:], ps[:])
            nc.sync.dma_start(out=out.ap(), in_=res[:])

    nc.compile()
    return nc


_NC_CACHE = None


def _get_module():
    global _NC_CACHE
    if _NC_CACHE is None:
        _NC_CACHE = build_module()
    return _NC_CACHE


def make_in_maps(x, x_out, y, mu, disc_pos):
    x = np.ascontiguousarray(x, dtype=np.float32)
    x_out = np.ascontiguousarray(x_out, dtype=np.float32)
    y = np.ascontiguousarray(y, dtype=np.float32)
    mu = np.ascontiguousarray(mu, dtype=np.float32)
    disc_pos = np.ascontiguousarray(disc_pos, dtype=np.float32)

    iota10 = np.arange(NPOS, dtype=np.float32)
    posb30 = np.tile(disc_pos, (BL, ND))
    iota30 = np.tile(iota10, (BL, ND))
    iota40 = np.tile(iota10, (BL, NL))

    in_maps = []
    for i in range(N_CORES):
        s = slice(i * BL, (i + 1) * BL)
        xcore = np.empty((2, TOT), dtype=np.float32)
        xcore[0] = x[s].reshape(-1)
        xcore[1] = x_out[s].reshape(-1)

        mu_s, y_s = mu[s], y[s]
        mu_d, mu_l = mu_s[:, :ND], mu_s[:, ND : ND + NL]
        y_d, y_l = y_s[:, :ND], y_s[:, ND : ND + NL]
        mu_rep30 = np.repeat(mu_d, NPOS, axis=1)
        y_rep70 = np.concatenate(
            [np.repeat(y_d, NPOS, axis=1), np.repeat(y_l, NPOS, axis=1)], axis=1
        )
        smalls = np.concatenate(
            [
                mu_rep30,           # [0:30)
                posb30,             # [30:60)
                iota40,             # [60:100)
                iota30,             # [100:130)
                iota40,             # [130:170)
                y_rep70,            # [170:240)
                mu_l,               # [240:244)
                y_l,                # [244:248)
                y_s[:, : ND + NL],  # [248:255)
                np.zeros((BL, 1), dtype=np.float32),  # pad to 256
            ],
            axis=1,
        ).astype(np.float32)
        assert smalls.shape == (BL, SM_W)
        in_maps.append({"xc": xcore, "smalls": smalls})
    return in_maps


def combine_partials(partials):
    """partials: [8, 1, 3] (or [8, 3]) per-core sums -> full (3,) output."""
    p = np.asarray(partials, dtype=np.float64).reshape(N_CORES, 3)
    s = p.sum(axis=0) / B
    recon = s[0]
    kld = s[1] + s[2]
    return np.array([recon, kld, recon + kld], dtype=np.float32)


def run_spmd(x, x_out, y, mu, disc_pos, trace=False, **kw):
    from concourse.bass_utils import run_bass_kernel_spmd

    nc = _get_module()
    in_maps = make_in_maps(x, x_out, y, mu, disc_pos)
    r = run_bass_kernel_spmd(nc, in_maps, list(range(N_CORES)), trace=trace, **kw)
    partials = [r.results[i]["out"] for i in range(N_CORES)]
    return combine_partials(partials), r


def kernel(x, x_out, y, mu, disc_pos):
    out, _ = run_spmd(x, x_out, y, mu, disc_pos)
    return out


if __name__ == "__main__":
    nc = build_module()
    print("module built ok")
